# revision 27
# baseline (speedup 1.0000x reference)
"""MoE gate (sigmoid scores + grouped top-k routing) on 8 Trainium2 cores.

Reference computation (per token):
    scores = sigmoid(x @ W.T)                  # [T, 256]
    s = scores + bias                          # selection scores
    group_score[g] = sum(top2(s[g*32:(g+1)*32]))
    keep top-4 groups, mask the rest to -inf
    idx = top8(masked s)                       # [T, 8] int32, descending
    w = scores[idx]; w = w / w.sum() * 2.5     # [T, 8] f32

Sharding: tokens split 8 ways (2048/core); W/bias replicated. Host
pre-shuffles x and W into the transposed tiled layout the TensorE needs
(contraction dim on partitions) so the device does no transposes.

Matmul: fp16 main pass at a 2^-6/2^6 scale split (xf = fp16(x*2^-6),
wh = fp16(w*2^6), product natural scale) plus ONE fused fp8e5m2
DoubleRow correction pass carrying both first-order residuals at
natural output scale:
    [xl5 ; xh5] x [wh5 ; wl5]
      = e5m2(x - 2^6 xf)*e5m2(w) + e5m2(x*2^-6)*e5m2(w*2^6 - fp16(w*2^6))
Both correction products land at natural scale, so the DR pass
accumulates directly into the main PSUM: no second PSUM, no ScalarE
downscale, no DVE combine. The scale split makes xh5 a plain dtype
cast of the resident fp16 x tile (gpsimd, per tile) and wh5 a cheap
scaled cast of the resident fp16 weights (DVE, once, during the
startup dead zone) -- cutting DMA to 3B/elem for x and 3B/elem for w:
~48MB total/core, well under the ~192us PE floor of the two passes.
Effective logit precision ~14 bits (idx metric ~1.1e-2 vs the 2e-2
gate; sim-validated against the fp32 reference).

Routing per 128-token tile, all on VectorE except the sigmoid
(ScalarE): grouped top-2 via segmented reduce_max + match_replace,
group top-4 via max8 threshold, expert top-8 via max8/find_index8.
The original-score gather uses a composite key u = 2^13*smask + orig:
max8(u) is ordered by smask (ties closer than ~2^-13 aside), so
orig[top8] = max8(u) - 2^13*max8(smask) elementwise -- replacing the
8-pass compare-accumulate gather with two cheap ops (w err ~3e-4).

Outputs w8 (f32) and idx (i32) are packed into one [128,16] u32 tile
per token tile and split on host, halving output DMA issues.

Software pipeline (steady state, step s):
    DMA     xf(s+2), xl5(s+1)     merged 3D-AP issues, ~2KB lines,
                                  sync/scalar queues alternated
    GpSimd  xh5(s) cast
    PE      main(s), corr(s-2), routing(s-2)
"""

import os

import numpy as np

import concourse.bass as bass
import concourse.mybir as mybir
import concourse.tile as tile
from concourse import bacc
from concourse.bass_utils import run_bass_kernel_spmd

T = 16384
DIM = 7168
E = 256
G = 8
EPG = E // G          # 32 experts per group
TOPKG = 4
TOPK = 8
SCALE = 2.5
NCORES = 8
TPC = T // NCORES     # 2048 tokens per core
P = 128
NT = TPC // P         # 16 token tiles per core
KT = DIM // P         # 56 contraction tiles
NEG = -1.0e30

XS = 2.0 ** -4        # host: xf = fp16(x * XS), wh = fp16(w / XS)
BIG = 8192.0          # composite-gather key scale (2^13)

DEFER = int(os.environ.get("GATE_KERNEL_DEFER", "3"))
# engine for the per-tile fp16 -> e5m2 x cast: gpsimd | vector | split
# (gpsimd's Q7 ucode casts at ~3.4ns/elem and its SBUF traffic stalls
# concurrent DVE ops; DVE casts at ~0.7ns/elem)
CAST_ENG = os.environ.get("GATE_KERNEL_CAST", "vector")
# 256-wide warm matmuls fill the PE until the wave-1 DMA completes
# (~24us with the HBM ramp): the clock gate never drops and the PE
# transitions straight into tile-0's matmuls
NWARM = int(os.environ.get("GATE_KERNEL_NWARM", "120"))

f32 = mybir.dt.float32
f16 = mybir.dt.float16
bf16 = mybir.dt.bfloat16
f8e5 = mybir.dt.float8e5
i32 = mybir.dt.int32
u16 = mybir.dt.uint16
u32 = mybir.dt.uint32
Alu = mybir.AluOpType
Act = mybir.ActivationFunctionType
AxX = mybir.AxisListType.X
DRow = mybir.MatmulPerfMode.DoubleRow

last_run = {}


def _build():
    nc = bacc.Bacc("TRN2", target_bir_lowering=False, debug=False,
                   num_devices=NCORES)

    xt = nc.dram_tensor("xt", [NT, P, DIM], f16, kind="ExternalInput").ap()
    xlo = nc.dram_tensor("xlo", [NT, P, DIM], f8e5, kind="ExternalInput").ap()
    wt = nc.dram_tensor("wt", [P, KT * E], f16, kind="ExternalInput").ap()
    wl = nc.dram_tensor("wl", [P, KT * E], f8e5, kind="ExternalInput").ap()
    biasb = nc.dram_tensor("biasb", [P, E], f32, kind="ExternalInput").ap()
    out = nc.dram_tensor("out", [TPC, 2 * TOPK], u32, kind="ExternalOutput").ap()

    with tile.TileContext(nc) as tc:
        with (
            tc.tile_pool(name="const", bufs=1) as const,
            tc.tile_pool(name="xp", bufs=4) as xp,
            tc.tile_pool(name="xcp", bufs=5) as xcp,
            tc.tile_pool(name="ps", bufs=5, space="PSUM") as psp,
            tc.tile_pool(name="rt", bufs=3) as rt,
        ):
            # PE warmup: dummy matmuls on zeroed scratch with no DMA deps —
            # holds the HAM clock gate at full speed while the first
            # operands stream in
            warm_sb = const.tile([P, 2 * P], bf16, tag="warm")
            nc.vector.memset(warm_sb[:], 0.0)
            with tc.tile_pool(name="warmps", bufs=1, space="PSUM") as wpsp:
                warm_ps = wpsp.tile([P, 2 * P], f32)
                for i in range(NWARM):
                    nc.tensor.matmul(warm_ps[:], warm_sb[:, 0:P], warm_sb[:],
                                     start=(i == 0), stop=(i == NWARM - 1))

            wt_t = const.tile([P, KT * E], f16)
            # correction rhs: [0:KT*E] = wh5 (derived on DVE), rest = wl5
            wc_t = const.tile([P, 2 * KT * E], f8e5)
            bias_t = const.tile([P, E], f32)
            engs = [nc.sync, nc.scalar]
            xf, xc, pss = {}, {}, {}

            # --- startup DMA waves ---
            # Few, escalating issues: the HWDGE issue cost (~0.65us each)
            # otherwise gates the whole startup. k-space is split in half
            # across the two queues so both halves stream in parallel
            # (the PE's k-order sweep only needs k=0 first).
            xf[0] = xp.tile([P, DIM], f16, tag="x", name="xf0")
            KH = KT // 2
            ESC = [(0, 8), (8, KH)]  # escalating k-chunks
            # wave 1: wt x xf0
            for qi in (0, 1):
                for a, b in ESC:
                    ka, kb = qi * KH + a, qi * KH + b
                    engs[qi].dma_start(wt_t[:, ka * E:kb * E],
                                       wt[:, ka * E:kb * E],
                                       max_dma_last_dim=1024)
                    engs[qi].dma_start(xf[0][:, ka * P:kb * P],
                                       xt[0][:, ka * P:kb * P],
                                       max_dma_last_dim=1024)
            # wh5 = e5m2(wt * XS), derived as wt lands; DVE is idle until
            # the first routing
            for qi in (0, 1):
                for a, b in ESC:
                    ka, kb = qi * KH + a, qi * KH + b
                    nc.vector.tensor_scalar(
                        wc_t[:, ka * E:kb * E],
                        wt_t[:, ka * E:kb * E], XS, None, op0=Alu.mult)
            # wave 2: xf1..xf3 (half-split across queues) — with DEFER=3
            # the PE consumes mains 0..3 before the first correction, so
            # all their x tiles outrank the correction operands
            H = DIM // 2
            for t in (1, 2, 3):
                xf[t] = xp.tile([P, DIM], f16, tag="x", name=f"xf{t}")
                engs[t % 2].dma_start(xf[t][:, 0:H], xt[t][:, 0:H],
                                      max_dma_last_dim=1024)
                engs[(t + 1) % 2].dma_start(xf[t][:, H:DIM], xt[t][:, H:DIM],
                                            max_dma_last_dim=1024)
            nc.sync.dma_start(bias_t[:], biasb)
            # wave 3: wl5 + xl5(0) (correction operands)
            xc[0] = xcp.tile([P, 2 * DIM], f8e5, tag="xc", name="xc0")
            nc.sync.dma_start(wc_t[:, KT * E:(KT + KH) * E],
                              wl[:, 0:KH * E], max_dma_last_dim=2048)
            nc.scalar.dma_start(wc_t[:, (KT + KH) * E:2 * KT * E],
                                wl[:, KH * E:KT * E], max_dma_last_dim=2048)
            nc.sync.dma_start(xc[0][:, 0:H], xlo[0][:, 0:H],
                              max_dma_last_dim=2048)
            nc.scalar.dma_start(xc[0][:, H:DIM], xlo[0][:, H:DIM],
                                max_dma_last_dim=2048)

            def cast_xh5(t):
                # xh5 = e5m2(xf) into the upper half of the xc tile; the
                # scale split makes this a plain dtype cast
                if CAST_ENG == "gpsimd":
                    nc.gpsimd.tensor_copy(xc[t][:, DIM:2 * DIM],
                                          xf[t][:, 0:DIM])
                elif CAST_ENG == "vector":
                    nc.vector.tensor_copy(xc[t][:, DIM:2 * DIM],
                                          xf[t][:, 0:DIM])
                else:
                    h = DIM // 2
                    nc.gpsimd.tensor_copy(xc[t][:, DIM:DIM + h],
                                          xf[t][:, 0:h])
                    nc.vector.tensor_copy(xc[t][:, DIM + h:2 * DIM],
                                          xf[t][:, h:DIM])

            def routing(tt, ps):
                # ---- sigmoid (PSUM -> SBUF) ----
                orig = rt.tile([P, E], f32, tag="orig")
                nc.scalar.activation(orig[:], ps[:], Act.Sigmoid)

                # ---- selection scores s = orig + bias ----
                s = rt.tile([P, E], f32, tag="s")
                nc.vector.tensor_tensor(s[:], orig[:], bias_t[:], Alu.add)
                s3 = s[:].rearrange("p (g j) -> p g j", g=G)

                # ---- per-group top-2 sum ----
                m1 = rt.tile([P, G], f32, tag="m1")
                nc.vector.tensor_reduce(m1[:], s3, AxX, Alu.max)
                srep = rt.tile([P, E], f32, tag="srep")
                nc.vector.match_replace(srep[:], m1[:], s[:], NEG)
                m2 = rt.tile([P, G], f32, tag="m2")
                nc.vector.tensor_reduce(
                    m2[:], srep[:].rearrange("p (g j) -> p g j", g=G),
                    AxX, Alu.max)
                gs = rt.tile([P, G], f32, tag="gs")
                nc.vector.tensor_tensor(gs[:], m1[:], m2[:], Alu.add)

                # ---- top-4 groups: threshold = 4th largest group score ----
                gtop = rt.tile([P, 8], f32, tag="gtop")
                nc.vector.max(gtop[:], gs[:])
                km = rt.tile([P, G], f32, tag="km")  # 0 kept, NEG dropped
                nc.vector.tensor_scalar(
                    km[:], gs[:], gtop[:, TOPKG - 1:TOPKG], NEG,
                    op0=Alu.is_lt, op1=Alu.mult)

                # ---- mask dropped groups in one op (broadcast km) ----
                smask = rt.tile([P, E], f32, tag="smask")
                kmb = km[:].unsqueeze(2).broadcast_to((P, G, EPG))
                nc.vector.tensor_tensor(
                    smask[:].rearrange("p (g j) -> p g j", g=G),
                    s3, kmb, Alu.add)

                # ---- expert top-8 values; composite-key gather of the
                # original scores (critical path to w8 runs first, the
                # index extraction fills in afterwards) ----
                v8 = rt.tile([P, TOPK], f32, tag="v8")
                nc.vector.max(v8[:], smask[:])
                u = rt.tile([P, E], f32, tag="u")
                nc.vector.scalar_tensor_tensor(
                    u[:], smask[:], BIG, orig[:], op0=Alu.mult, op1=Alu.add)
                u8 = rt.tile([P, TOPK], f32, tag="u8")
                nc.vector.max(u8[:], u[:])
                o8 = rt.tile([P, TOPK], f32, tag="o8")
                nc.vector.scalar_tensor_tensor(
                    o8[:], v8[:], -BIG, u8[:], op0=Alu.mult, op1=Alu.add)

                # ---- normalize * SCALE; pack [w8|i8] into one u32 tile ----
                ssum = rt.tile([P, 1], f32, tag="ssum")
                nc.vector.tensor_reduce(ssum[:], o8[:], AxX, Alu.add)
                rec = rt.tile([P, 1], f32, tag="rec")
                nc.vector.reciprocal(rec[:], ssum[:])
                ot = rt.tile([P, 2 * TOPK], u32, tag="ot")
                nc.vector.tensor_scalar(
                    ot[:, 0:TOPK].bitcast(f32), o8[:], rec[:, 0:1], SCALE,
                    op0=Alu.mult, op1=Alu.mult)
                i8u = rt.tile([P, TOPK], u16, tag="i8u")
                nc.vector.max_index(i8u[:], v8[:], smask[:])
                nc.vector.tensor_copy(ot[:, TOPK:2 * TOPK].bitcast(i32),
                                      i8u[:])
                nc.sync.dma_start(out[tt * P:(tt + 1) * P, :], ot[:])

            wc_pair = wc_t[:].rearrange("p (two d) -> p two d", two=2)

            def corr(t):
                # fused e5m2 DoubleRow correction into the main PSUM
                lhs_all = xc[t][:].rearrange("p (two d) -> p two d", two=2)
                ps = pss[t]
                for k in range(KT):
                    nc.tensor.matmul(
                        ps[:],
                        lhs_all[:, :, k * P:(k + 1) * P],
                        wc_pair[:, :, k * E:(k + 1) * E],
                        start=False, stop=(k == KT - 1),
                        perf_mode=DRow)
                del xc[t]

            # --- software-pipelined main loop ---
            for s in range(NT + DEFER):
                if s < NT:
                    if 0 < s and s + 3 < NT:
                        tp = s + 3
                        xf[tp] = xp.tile([P, DIM], f16, tag="x", name=f"xf{tp}")
                        engs[s % 2].dma_start(xf[tp][:, 0:H], xt[tp][:, 0:H],
                                              max_dma_last_dim=1024)
                        engs[(s + 1) % 2].dma_start(xf[tp][:, H:DIM],
                                                    xt[tp][:, H:DIM],
                                                    max_dma_last_dim=1024)
                    if 0 < s + 1 < NT:
                        tq = s + 1
                        xc[tq] = xcp.tile([P, 2 * DIM], f8e5, tag="xc", name=f"xc{tq}")
                        engs[s % 2].dma_start(xc[tq][:, 0:H], xlo[tq][:, 0:H],
                                              max_dma_last_dim=2048)
                        engs[(s + 1) % 2].dma_start(xc[tq][:, H:DIM],
                                                    xlo[tq][:, H:DIM],
                                                    max_dma_last_dim=2048)
                    # cast leads the step: xf(s) is already resident
                    # (prefetched 3 tiles ahead) and corr(s) is DEFER
                    # periods away, so the DVE cast is far off the
                    # critical path
                    cast_xh5(s)
                    ps = psp.tile([P, E], f32)
                    pss[s] = ps
                    for k in range(KT):
                        nc.tensor.matmul(ps[:], xf[s][:, k * P:(k + 1) * P],
                                         wt_t[:, k * E:(k + 1) * E],
                                         start=(k == 0), stop=False)
                    del xf[s]
                if s >= DEFER:
                    t = s - DEFER
                    corr(t)
                    routing(t, pss.pop(t))

    nc.compile()
    return nc


def _shuffle_x(xc):
    """[TPC, DIM] -> [NT, P, DIM] with out[tt, p, k*128+j] = xc[tt*128+j, k*128+p]."""
    return np.ascontiguousarray(
        xc.reshape(NT, P, KT, P).transpose(0, 3, 2, 1).reshape(NT, P, DIM))


def _shuffle_w(w):
    """[E, DIM] -> [P, KT*E] with out[p, k*E+e] = w[e, k*128+p]."""
    return np.ascontiguousarray(
        w.T.reshape(KT, P, E).transpose(1, 0, 2).reshape(P, KT * E))


_nc_cache = {}


def kernel(x, weight, bias):
    import ml_dtypes

    f8 = ml_dtypes.float8_e5m2
    x = np.asarray(x, dtype=np.float32)
    weight = np.asarray(weight, dtype=np.float32)
    bias = np.asarray(bias, dtype=np.float32)

    if "nc" not in _nc_cache:
        _nc_cache["nc"] = _build()
    nc = _nc_cache["nc"]

    biasb = np.ascontiguousarray(np.broadcast_to(bias, (P, E)))
    # weight: fp16 main in the 2^6 domain; e5m2 residual in the same domain
    w6 = weight / np.float32(XS)
    wh16 = w6.astype(np.float16)
    wl5 = (w6 - wh16.astype(np.float32))
    wt_h = _shuffle_w(wh16.astype(np.float32)).astype(np.float16)
    wl_h = _shuffle_w(wl5).astype(f8)

    in_maps = []
    for c in range(NCORES):
        xcore = x[c * TPC:(c + 1) * TPC]
        xf16 = (xcore * np.float32(XS)).astype(np.float16)
        xl = xcore - xf16.astype(np.float32) / np.float32(XS)
        in_maps.append({
            "xt": _shuffle_x(xf16.astype(np.float32)).astype(np.float16),
            "xlo": _shuffle_x(xl).astype(f8),
            "wt": wt_h, "wl": wl_h, "biasb": biasb,
        })

    trace = bool(int(os.environ.get("GATE_KERNEL_TRACE", "0")))
    res = run_bass_kernel_spmd(nc, in_maps, core_ids=list(range(NCORES)),
                               trace=trace)
    last_run["exec_time_ns"] = res.exec_time_ns
    last_run["mean_exec_time_ns"] = res.mean_exec_time_ns
    last_run["trace"] = res.instructions_and_trace

    outs = [res.results[c]["out"] for c in range(NCORES)]
    buf = np.concatenate(outs, axis=0)
    w8 = buf[:, 0:TOPK].view(np.float32)
    idx = buf[:, TOPK:2 * TOPK].view(np.int32)
    return np.ascontiguousarray(w8), np.ascontiguousarray(idx)


# revision 28
# speedup vs baseline: 1.1094x; 1.1094x over previous
"""MoE gate (sigmoid scores + grouped top-k routing) on 8 Trainium2 cores.

Reference computation (per token):
    scores = sigmoid(x @ W.T)                  # [T, 256]
    s = scores + bias                          # selection scores
    group_score[g] = sum(top2(s[g*32:(g+1)*32]))
    keep top-4 groups, mask the rest to -inf
    idx = top8(masked s)                       # [T, 8] int32, descending
    w = scores[idx]; w = w / w.sum() * 2.5     # [T, 8] f32

Sharding: tokens split 8 ways (2048/core); W/bias replicated. Host
pre-shuffles x and W into the transposed tiled layout the TensorE needs
(contraction dim on partitions) so the device does no transposes.

Matmul: fp16 main pass at a 2^-6/2^6 scale split (xf = fp16(x*2^-6),
wh = fp16(w*2^6), product natural scale) plus ONE fused fp8e5m2
DoubleRow correction pass carrying both first-order residuals at
natural output scale:
    [xl5 ; xh5] x [wh5 ; wl5]
      = e5m2(x - 2^6 xf)*e5m2(w) + e5m2(x*2^-6)*e5m2(w*2^6 - fp16(w*2^6))
Both correction products land at natural scale, so the DR pass
accumulates directly into the main PSUM: no second PSUM, no ScalarE
downscale, no DVE combine. The scale split makes xh5 a plain dtype
cast of the resident fp16 x tile (gpsimd, per tile) and wh5 a cheap
scaled cast of the resident fp16 weights (DVE, once, during the
startup dead zone) -- cutting DMA to 3B/elem for x and 3B/elem for w:
~48MB total/core, well under the ~192us PE floor of the two passes.
Effective logit precision ~14 bits (idx metric ~1.1e-2 vs the 2e-2
gate; sim-validated against the fp32 reference).

Routing per 128-token tile, all on VectorE except the sigmoid
(ScalarE): grouped top-2 via segmented reduce_max + match_replace,
group top-4 via max8 threshold, expert top-8 via max8/find_index8.
The original-score gather uses a composite key u = 2^13*smask + orig:
max8(u) is ordered by smask (ties closer than ~2^-13 aside), so
orig[top8] = max8(u) - 2^13*max8(smask) elementwise -- replacing the
8-pass compare-accumulate gather with two cheap ops (w err ~3e-4).

Outputs w8 (f32) and idx (i32) are packed into one [128,16] u32 tile
per token tile and split on host, halving output DMA issues.

Software pipeline (steady state, step s):
    DMA     xf(s+2), xl5(s+1)     merged 3D-AP issues, ~2KB lines,
                                  sync/scalar queues alternated
    GpSimd  xh5(s) cast
    PE      main(s), corr(s-2), routing(s-2)
"""

import os

import numpy as np

import concourse.bass as bass
import concourse.mybir as mybir
import concourse.tile as tile
from concourse import bacc
from concourse.bass_utils import run_bass_kernel_spmd

T = 16384
DIM = 7168
E = 256
G = 8
EPG = E // G          # 32 experts per group
TOPKG = 4
TOPK = 8
SCALE = 2.5
NCORES = 8
TPC = T // NCORES     # 2048 tokens per core
P = 128
NT = TPC // P         # 16 token tiles per core
KT = DIM // P         # 56 contraction tiles
NEG = -1.0e30

XS = 2.0 ** -4        # host: xf = fp16(x * XS), wh = fp16(w / XS)
BIG = 8192.0          # composite-gather key scale (2^13)

DEFER = int(os.environ.get("GATE_KERNEL_DEFER", "3"))
# engine for the per-tile fp16 -> e5m2 x cast: gpsimd | vector | split
# (gpsimd's Q7 ucode casts at ~3.4ns/elem and its SBUF traffic stalls
# concurrent DVE ops; DVE casts at ~0.7ns/elem)
CAST_ENG = os.environ.get("GATE_KERNEL_CAST", "vector")
# 256-wide warm matmuls fill the PE until the wave-1 DMA completes
# (~24us with the HBM ramp): the clock gate never drops and the PE
# transitions straight into tile-0's matmuls
NWARM = int(os.environ.get("GATE_KERNEL_NWARM", "120"))

f32 = mybir.dt.float32
f16 = mybir.dt.float16
bf16 = mybir.dt.bfloat16
f8e5 = mybir.dt.float8e5
i32 = mybir.dt.int32
u16 = mybir.dt.uint16
u32 = mybir.dt.uint32
Alu = mybir.AluOpType
Act = mybir.ActivationFunctionType
AxX = mybir.AxisListType.X
DRow = mybir.MatmulPerfMode.DoubleRow

last_run = {}


def _build():
    nc = bacc.Bacc("TRN2", target_bir_lowering=False, debug=False,
                   num_devices=NCORES)

    xt = nc.dram_tensor("xt", [NT, P, DIM], f16, kind="ExternalInput").ap()
    xlo = nc.dram_tensor("xlo", [NT, P, DIM], f8e5, kind="ExternalInput").ap()
    wt = nc.dram_tensor("wt", [P, KT * E], f16, kind="ExternalInput").ap()
    wl = nc.dram_tensor("wl", [P, KT * E], f8e5, kind="ExternalInput").ap()
    biasb = nc.dram_tensor("biasb", [P, E], f32, kind="ExternalInput").ap()
    out = nc.dram_tensor("out", [TPC, 2 * TOPK], u32, kind="ExternalOutput").ap()

    with tile.TileContext(nc) as tc:
        with (
            tc.tile_pool(name="const", bufs=1) as const,
            tc.tile_pool(name="xp", bufs=4) as xp,
            tc.tile_pool(name="xcp", bufs=5) as xcp,
            tc.tile_pool(name="ps", bufs=5, space="PSUM") as psp,
            tc.tile_pool(name="rt", bufs=3) as rt,
        ):
            # PE warmup: dummy matmuls on zeroed scratch with no DMA deps —
            # holds the HAM clock gate at full speed while the first
            # operands stream in
            warm_sb = const.tile([P, 2 * P], bf16, tag="warm")
            nc.vector.memset(warm_sb[:], 0.0)
            with tc.tile_pool(name="warmps", bufs=1, space="PSUM") as wpsp:
                warm_ps = wpsp.tile([P, 2 * P], f32)
                for i in range(NWARM):
                    nc.tensor.matmul(warm_ps[:], warm_sb[:, 0:P], warm_sb[:],
                                     start=(i == 0), stop=(i == NWARM - 1))

            wt_t = const.tile([P, KT * E], f16)
            # correction rhs: [0:KT*E] = wh5 (derived on DVE), rest = wl5
            wc_t = const.tile([P, 2 * KT * E], f8e5)
            bias_t = const.tile([P, E], f32)
            engs = [nc.sync, nc.scalar]
            xf, xc, pss = {}, {}, {}

            # --- startup DMA waves ---
            # Few, escalating issues: the HWDGE issue cost (~0.65us each)
            # otherwise gates the whole startup. k-space is split in half
            # across the two queues so both halves stream in parallel
            # (the PE's k-order sweep only needs k=0 first).
            xf[0] = xp.tile([P, DIM], f16, tag="x", name="xf0")
            KH = KT // 2
            ESC = [(0, 8), (8, KH)]  # escalating k-chunks
            # wave 1: wt x xf0. The first chunk of each k-half goes out on
            # gpsimd's SWDGE: its engine preamble retires first, so these
            # transfers are in flight ~2us before the HWDGE queues wake
            for qi in (0, 1):
                for ei, (a, b) in enumerate(ESC):
                    ka, kb = qi * KH + a, qi * KH + b
                    eng = nc.gpsimd if ei == 0 else engs[qi]
                    eng.dma_start(wt_t[:, ka * E:kb * E],
                                  wt[:, ka * E:kb * E],
                                  max_dma_last_dim=1024)
                    eng.dma_start(xf[0][:, ka * P:kb * P],
                                  xt[0][:, ka * P:kb * P],
                                  max_dma_last_dim=1024)
            # wh5 = e5m2(wt * XS), derived as wt lands; DVE is idle until
            # the first routing
            for qi in (0, 1):
                for a, b in ESC:
                    ka, kb = qi * KH + a, qi * KH + b
                    nc.vector.tensor_scalar(
                        wc_t[:, ka * E:kb * E],
                        wt_t[:, ka * E:kb * E], XS, None, op0=Alu.mult)
            # wave 2: xf1..xf3 (half-split across queues) — with DEFER=3
            # the PE consumes mains 0..3 before the first correction, so
            # all their x tiles outrank the correction operands
            H = DIM // 2
            for t in (1, 2, 3):
                xf[t] = xp.tile([P, DIM], f16, tag="x", name=f"xf{t}")
                engs[t % 2].dma_start(xf[t][:, 0:H], xt[t][:, 0:H],
                                      max_dma_last_dim=1024)
                engs[(t + 1) % 2].dma_start(xf[t][:, H:DIM], xt[t][:, H:DIM],
                                            max_dma_last_dim=1024)
            nc.sync.dma_start(bias_t[:], biasb)
            # wave 3: wl5 + xl5(0) (correction operands)
            xc[0] = xcp.tile([P, 2 * DIM], f8e5, tag="xc", name="xc0")
            nc.sync.dma_start(wc_t[:, KT * E:(KT + KH) * E],
                              wl[:, 0:KH * E], max_dma_last_dim=2048)
            nc.scalar.dma_start(wc_t[:, (KT + KH) * E:2 * KT * E],
                                wl[:, KH * E:KT * E], max_dma_last_dim=2048)
            nc.sync.dma_start(xc[0][:, 0:H], xlo[0][:, 0:H],
                              max_dma_last_dim=2048)
            nc.scalar.dma_start(xc[0][:, H:DIM], xlo[0][:, H:DIM],
                                max_dma_last_dim=2048)

            def cast_xh5(t):
                # xh5 = e5m2(xf) into the upper half of the xc tile; the
                # scale split makes this a plain dtype cast
                if CAST_ENG == "gpsimd":
                    nc.gpsimd.tensor_copy(xc[t][:, DIM:2 * DIM],
                                          xf[t][:, 0:DIM])
                elif CAST_ENG == "vector":
                    nc.vector.tensor_copy(xc[t][:, DIM:2 * DIM],
                                          xf[t][:, 0:DIM])
                else:
                    h = DIM // 2
                    nc.gpsimd.tensor_copy(xc[t][:, DIM:DIM + h],
                                          xf[t][:, 0:h])
                    nc.vector.tensor_copy(xc[t][:, DIM + h:2 * DIM],
                                          xf[t][:, h:DIM])

            def routing(tt, ps):
                # ---- sigmoid (PSUM -> SBUF) ----
                orig = rt.tile([P, E], f32, tag="orig")
                nc.scalar.activation(orig[:], ps[:], Act.Sigmoid)

                # ---- selection scores s = orig + bias ----
                s = rt.tile([P, E], f32, tag="s")
                nc.vector.tensor_tensor(s[:], orig[:], bias_t[:], Alu.add)
                s3 = s[:].rearrange("p (g j) -> p g j", g=G)

                # ---- per-group top-2 sum ----
                m1 = rt.tile([P, G], f32, tag="m1")
                nc.vector.tensor_reduce(m1[:], s3, AxX, Alu.max)
                srep = rt.tile([P, E], f32, tag="srep")
                nc.vector.match_replace(srep[:], m1[:], s[:], NEG)
                m2 = rt.tile([P, G], f32, tag="m2")
                nc.vector.tensor_reduce(
                    m2[:], srep[:].rearrange("p (g j) -> p g j", g=G),
                    AxX, Alu.max)
                gs = rt.tile([P, G], f32, tag="gs")
                nc.vector.tensor_tensor(gs[:], m1[:], m2[:], Alu.add)

                # ---- top-4 groups: threshold = 4th largest group score ----
                gtop = rt.tile([P, 8], f32, tag="gtop")
                nc.vector.max(gtop[:], gs[:])
                km = rt.tile([P, G], f32, tag="km")  # 0 kept, NEG dropped
                nc.vector.tensor_scalar(
                    km[:], gs[:], gtop[:, TOPKG - 1:TOPKG], NEG,
                    op0=Alu.is_lt, op1=Alu.mult)

                # ---- mask dropped groups in one op (broadcast km) ----
                smask = rt.tile([P, E], f32, tag="smask")
                kmb = km[:].unsqueeze(2).broadcast_to((P, G, EPG))
                nc.vector.tensor_tensor(
                    smask[:].rearrange("p (g j) -> p g j", g=G),
                    s3, kmb, Alu.add)

                # ---- expert top-8 values; composite-key gather of the
                # original scores (critical path to w8 runs first, the
                # index extraction fills in afterwards) ----
                v8 = rt.tile([P, TOPK], f32, tag="v8")
                nc.vector.max(v8[:], smask[:])
                u = rt.tile([P, E], f32, tag="u")
                nc.vector.scalar_tensor_tensor(
                    u[:], smask[:], BIG, orig[:], op0=Alu.mult, op1=Alu.add)
                u8 = rt.tile([P, TOPK], f32, tag="u8")
                nc.vector.max(u8[:], u[:])
                o8 = rt.tile([P, TOPK], f32, tag="o8")
                nc.vector.scalar_tensor_tensor(
                    o8[:], v8[:], -BIG, u8[:], op0=Alu.mult, op1=Alu.add)

                # ---- normalize * SCALE; pack [w8|i8] into one u32 tile ----
                ssum = rt.tile([P, 1], f32, tag="ssum")
                nc.vector.tensor_reduce(ssum[:], o8[:], AxX, Alu.add)
                rec = rt.tile([P, 1], f32, tag="rec")
                nc.vector.reciprocal(rec[:], ssum[:])
                ot = rt.tile([P, 2 * TOPK], u32, tag="ot")
                nc.vector.tensor_scalar(
                    ot[:, 0:TOPK].bitcast(f32), o8[:], rec[:, 0:1], SCALE,
                    op0=Alu.mult, op1=Alu.mult)
                i8u = rt.tile([P, TOPK], u16, tag="i8u")
                nc.vector.max_index(i8u[:], v8[:], smask[:])
                nc.vector.tensor_copy(ot[:, TOPK:2 * TOPK].bitcast(i32),
                                      i8u[:])
                nc.sync.dma_start(out[tt * P:(tt + 1) * P, :], ot[:])

            wc_pair = wc_t[:].rearrange("p (two d) -> p two d", two=2)

            def corr(t):
                # fused e5m2 DoubleRow correction into the main PSUM
                lhs_all = xc[t][:].rearrange("p (two d) -> p two d", two=2)
                ps = pss[t]
                for k in range(KT):
                    nc.tensor.matmul(
                        ps[:],
                        lhs_all[:, :, k * P:(k + 1) * P],
                        wc_pair[:, :, k * E:(k + 1) * E],
                        start=False, stop=(k == KT - 1),
                        perf_mode=DRow)
                del xc[t]

            # --- software-pipelined main loop ---
            for s in range(NT + DEFER):
                if s < NT:
                    if 0 < s and s + 3 < NT:
                        tp = s + 3
                        xf[tp] = xp.tile([P, DIM], f16, tag="x", name=f"xf{tp}")
                        engs[s % 2].dma_start(xf[tp][:, 0:H], xt[tp][:, 0:H],
                                              max_dma_last_dim=1024)
                        engs[(s + 1) % 2].dma_start(xf[tp][:, H:DIM],
                                                    xt[tp][:, H:DIM],
                                                    max_dma_last_dim=1024)
                    if 0 < s + 1 < NT:
                        tq = s + 1
                        xc[tq] = xcp.tile([P, 2 * DIM], f8e5, tag="xc", name=f"xc{tq}")
                        engs[s % 2].dma_start(xc[tq][:, 0:H], xlo[tq][:, 0:H],
                                              max_dma_last_dim=2048)
                        engs[(s + 1) % 2].dma_start(xc[tq][:, H:DIM],
                                                    xlo[tq][:, H:DIM],
                                                    max_dma_last_dim=2048)
                    # cast leads the step: xf(s) is already resident
                    # (prefetched 3 tiles ahead) and corr(s) is DEFER
                    # periods away, so the DVE cast is far off the
                    # critical path
                    cast_xh5(s)
                    ps = psp.tile([P, E], f32)
                    pss[s] = ps
                    for k in range(KT):
                        nc.tensor.matmul(ps[:], xf[s][:, k * P:(k + 1) * P],
                                         wt_t[:, k * E:(k + 1) * E],
                                         start=(k == 0), stop=False)
                    del xf[s]
                if s >= DEFER:
                    t = s - DEFER
                    corr(t)
                    routing(t, pss.pop(t))

    nc.compile()
    return nc


def _shuffle_x(xc):
    """[TPC, DIM] -> [NT, P, DIM] with out[tt, p, k*128+j] = xc[tt*128+j, k*128+p]."""
    return np.ascontiguousarray(
        xc.reshape(NT, P, KT, P).transpose(0, 3, 2, 1).reshape(NT, P, DIM))


def _shuffle_w(w):
    """[E, DIM] -> [P, KT*E] with out[p, k*E+e] = w[e, k*128+p]."""
    return np.ascontiguousarray(
        w.T.reshape(KT, P, E).transpose(1, 0, 2).reshape(P, KT * E))


_nc_cache = {}


def kernel(x, weight, bias):
    import ml_dtypes

    f8 = ml_dtypes.float8_e5m2
    x = np.asarray(x, dtype=np.float32)
    weight = np.asarray(weight, dtype=np.float32)
    bias = np.asarray(bias, dtype=np.float32)

    if "nc" not in _nc_cache:
        _nc_cache["nc"] = _build()
    nc = _nc_cache["nc"]

    biasb = np.ascontiguousarray(np.broadcast_to(bias, (P, E)))
    # weight: fp16 main in the 2^6 domain; e5m2 residual in the same domain
    w6 = weight / np.float32(XS)
    wh16 = w6.astype(np.float16)
    wl5 = (w6 - wh16.astype(np.float32))
    wt_h = _shuffle_w(wh16.astype(np.float32)).astype(np.float16)
    wl_h = _shuffle_w(wl5).astype(f8)

    in_maps = []
    for c in range(NCORES):
        xcore = x[c * TPC:(c + 1) * TPC]
        xf16 = (xcore * np.float32(XS)).astype(np.float16)
        xl = xcore - xf16.astype(np.float32) / np.float32(XS)
        in_maps.append({
            "xt": _shuffle_x(xf16.astype(np.float32)).astype(np.float16),
            "xlo": _shuffle_x(xl).astype(f8),
            "wt": wt_h, "wl": wl_h, "biasb": biasb,
        })

    trace = bool(int(os.environ.get("GATE_KERNEL_TRACE", "0")))
    res = run_bass_kernel_spmd(nc, in_maps, core_ids=list(range(NCORES)),
                               trace=trace)
    last_run["exec_time_ns"] = res.exec_time_ns
    last_run["mean_exec_time_ns"] = res.mean_exec_time_ns
    last_run["trace"] = res.instructions_and_trace

    outs = [res.results[c]["out"] for c in range(NCORES)]
    buf = np.concatenate(outs, axis=0)
    w8 = buf[:, 0:TOPK].view(np.float32)
    idx = buf[:, TOPK:2 * TOPK].view(np.int32)
    return np.ascontiguousarray(w8), np.ascontiguousarray(idx)


# revision 29
# speedup vs baseline: 1.1960x; 1.0781x over previous
"""MoE gate (sigmoid scores + grouped top-k routing) on 8 Trainium2 cores.

Reference computation (per token):
    scores = sigmoid(x @ W.T)                  # [T, 256]
    s = scores + bias                          # selection scores
    group_score[g] = sum(top2(s[g*32:(g+1)*32]))
    keep top-4 groups, mask the rest to -inf
    idx = top8(masked s)                       # [T, 8] int32, descending
    w = scores[idx]; w = w / w.sum() * 2.5     # [T, 8] f32

Sharding: tokens split 8 ways (2048/core); W/bias replicated. Host
pre-shuffles x and W into the transposed tiled layout the TensorE needs
(contraction dim on partitions) so the device does no transposes.

Matmul: fp16 main pass at a 2^-6/2^6 scale split (xf = fp16(x*2^-6),
wh = fp16(w*2^6), product natural scale) plus ONE fused fp8e5m2
DoubleRow correction pass carrying both first-order residuals at
natural output scale:
    [xl5 ; xh5] x [wh5 ; wl5]
      = e5m2(x - 2^6 xf)*e5m2(w) + e5m2(x*2^-6)*e5m2(w*2^6 - fp16(w*2^6))
Both correction products land at natural scale, so the DR pass
accumulates directly into the main PSUM: no second PSUM, no ScalarE
downscale, no DVE combine. The scale split makes xh5 a plain dtype
cast of the resident fp16 x tile (gpsimd, per tile) and wh5 a cheap
scaled cast of the resident fp16 weights (DVE, once, during the
startup dead zone) -- cutting DMA to 3B/elem for x and 3B/elem for w:
~48MB total/core, well under the ~192us PE floor of the two passes.
Effective logit precision ~14 bits (idx metric ~1.1e-2 vs the 2e-2
gate; sim-validated against the fp32 reference).

Routing per 128-token tile, all on VectorE except the sigmoid
(ScalarE): grouped top-2 via segmented reduce_max + match_replace,
group top-4 via max8 threshold, expert top-8 via max8/find_index8.
The original-score gather uses a composite key u = 2^13*smask + orig:
max8(u) is ordered by smask (ties closer than ~2^-13 aside), so
orig[top8] = max8(u) - 2^13*max8(smask) elementwise -- replacing the
8-pass compare-accumulate gather with two cheap ops (w err ~3e-4).

Outputs w8 (f32) and idx (i32) are packed into one [128,16] u32 tile
per token tile and split on host, halving output DMA issues.

Software pipeline (steady state, step s):
    DMA     xf(s+2), xl5(s+1)     merged 3D-AP issues, ~2KB lines,
                                  sync/scalar queues alternated
    GpSimd  xh5(s) cast
    PE      main(s), corr(s-2), routing(s-2)
"""

import os

import numpy as np

import concourse.bass as bass
import concourse.mybir as mybir
import concourse.tile as tile
from concourse import bacc
from concourse.bass_utils import run_bass_kernel_spmd

T = 16384
DIM = 7168
E = 256
G = 8
EPG = E // G          # 32 experts per group
TOPKG = 4
TOPK = 8
SCALE = 2.5
NCORES = 8
TPC = T // NCORES     # 2048 tokens per core
P = 128
NT = TPC // P         # 16 token tiles per core
KT = DIM // P         # 56 contraction tiles
NEG = -1.0e30

XS = 2.0 ** -4        # host: xf = fp16(x * XS), wh = fp16(w / XS)
BIG = 8192.0          # composite-gather key scale (2^13)

DEFER = int(os.environ.get("GATE_KERNEL_DEFER", "3"))
# engine for the per-tile fp16 -> e5m2 x cast: gpsimd | vector | split
# (gpsimd's Q7 ucode casts at ~3.4ns/elem and its SBUF traffic stalls
# concurrent DVE ops; DVE casts at ~0.7ns/elem)
CAST_ENG = os.environ.get("GATE_KERNEL_CAST", "vector")
# 256-wide warm matmuls fill the PE until the wave-1 DMA completes
# (~24us with the HBM ramp): the clock gate never drops and the PE
# transitions straight into tile-0's matmuls
NWARM = int(os.environ.get("GATE_KERNEL_NWARM", "120"))

f32 = mybir.dt.float32
f16 = mybir.dt.float16
bf16 = mybir.dt.bfloat16
f8e5 = mybir.dt.float8e5
i32 = mybir.dt.int32
u16 = mybir.dt.uint16
u32 = mybir.dt.uint32
Alu = mybir.AluOpType
Act = mybir.ActivationFunctionType
AxX = mybir.AxisListType.X
DRow = mybir.MatmulPerfMode.DoubleRow

last_run = {}


def _build():
    nc = bacc.Bacc("TRN2", target_bir_lowering=False, debug=False,
                   num_devices=NCORES)

    xt = nc.dram_tensor("xt", [NT, P, DIM], f16, kind="ExternalInput").ap()
    xlo = nc.dram_tensor("xlo", [NT, P, DIM], f8e5, kind="ExternalInput").ap()
    wt = nc.dram_tensor("wt", [P, KT * E], f16, kind="ExternalInput").ap()
    wl = nc.dram_tensor("wl", [P, KT * E], f8e5, kind="ExternalInput").ap()
    biasb = nc.dram_tensor("biasb", [P, E], f32, kind="ExternalInput").ap()
    out = nc.dram_tensor("out", [TPC, 2 * TOPK], u32, kind="ExternalOutput").ap()

    with tile.TileContext(nc) as tc:
        with (
            tc.tile_pool(name="const", bufs=1) as const,
            tc.tile_pool(name="xp", bufs=4) as xp,
            tc.tile_pool(name="xcp", bufs=5) as xcp,
            tc.tile_pool(name="ps", bufs=5, space="PSUM") as psp,
            tc.tile_pool(name="rt", bufs=3) as rt,
        ):
            # PE warmup: dummy matmuls on zeroed scratch with no DMA deps —
            # holds the HAM clock gate at full speed while the first
            # operands stream in
            warm_sb = const.tile([P, 2 * P], bf16, tag="warm")
            nc.vector.memset(warm_sb[:], 0.0)
            with tc.tile_pool(name="warmps", bufs=1, space="PSUM") as wpsp:
                warm_ps = wpsp.tile([P, 2 * P], f32)
                for i in range(NWARM):
                    nc.tensor.matmul(warm_ps[:], warm_sb[:, 0:P], warm_sb[:],
                                     start=(i == 0), stop=(i == NWARM - 1))

            wt_t = const.tile([P, KT * E], f16)
            # correction rhs: [0:KT*E] = wh5 (derived on DVE), rest = wl5
            wc_t = const.tile([P, 2 * KT * E], f8e5)
            bias_t = const.tile([P, E], f32)
            engs = [nc.sync, nc.scalar]
            xf, xc, pss = {}, {}, {}

            # --- startup DMA waves ---
            # Few, escalating issues: the HWDGE issue cost (~0.65us each)
            # otherwise gates the whole startup. k-space is split in half
            # across the two queues so both halves stream in parallel
            # (the PE's k-order sweep only needs k=0 first).
            xf[0] = xp.tile([P, DIM], f16, tag="x", name="xf0")
            KH = KT // 2
            ESC = [(0, 8), (8, KH)]  # escalating k-chunks
            # wave 1: wt x xf0
            for qi in (0, 1):
                for a, b in ESC:
                    ka, kb = qi * KH + a, qi * KH + b
                    engs[qi].dma_start(wt_t[:, ka * E:kb * E],
                                       wt[:, ka * E:kb * E],
                                       max_dma_last_dim=1024)
                    engs[qi].dma_start(xf[0][:, ka * P:kb * P],
                                       xt[0][:, ka * P:kb * P],
                                       max_dma_last_dim=1024)
            # wh5 = e5m2(wt * XS), derived as wt lands; DVE is idle until
            # the first routing
            for qi in (0, 1):
                for a, b in ESC:
                    ka, kb = qi * KH + a, qi * KH + b
                    nc.vector.tensor_scalar(
                        wc_t[:, ka * E:kb * E],
                        wt_t[:, ka * E:kb * E], XS, None, op0=Alu.mult)
            # wave 2: xf1..xf3 (half-split across queues) — with DEFER=3
            # the PE consumes mains 0..3 before the first correction, so
            # all their x tiles outrank the correction operands
            H = DIM // 2
            for t in (1, 2, 3):
                xf[t] = xp.tile([P, DIM], f16, tag="x", name=f"xf{t}")
                engs[t % 2].dma_start(xf[t][:, 0:H], xt[t][:, 0:H],
                                      max_dma_last_dim=1024)
                engs[(t + 1) % 2].dma_start(xf[t][:, H:DIM], xt[t][:, H:DIM],
                                            max_dma_last_dim=1024)
            nc.sync.dma_start(bias_t[:], biasb)
            # wave 3: wl5 + xl5(0) (correction operands)
            xc[0] = xcp.tile([P, 2 * DIM], f8e5, tag="xc", name="xc0")
            nc.sync.dma_start(wc_t[:, KT * E:(KT + KH) * E],
                              wl[:, 0:KH * E], max_dma_last_dim=2048)
            nc.scalar.dma_start(wc_t[:, (KT + KH) * E:2 * KT * E],
                                wl[:, KH * E:KT * E], max_dma_last_dim=2048)
            nc.sync.dma_start(xc[0][:, 0:H], xlo[0][:, 0:H],
                              max_dma_last_dim=2048)
            nc.scalar.dma_start(xc[0][:, H:DIM], xlo[0][:, H:DIM],
                                max_dma_last_dim=2048)

            def cast_xh5(t):
                # xh5 = e5m2(xf) into the upper half of the xc tile; the
                # scale split makes this a plain dtype cast
                if CAST_ENG == "gpsimd":
                    nc.gpsimd.tensor_copy(xc[t][:, DIM:2 * DIM],
                                          xf[t][:, 0:DIM])
                elif CAST_ENG == "vector":
                    nc.vector.tensor_copy(xc[t][:, DIM:2 * DIM],
                                          xf[t][:, 0:DIM])
                else:
                    h = DIM // 2
                    nc.gpsimd.tensor_copy(xc[t][:, DIM:DIM + h],
                                          xf[t][:, 0:h])
                    nc.vector.tensor_copy(xc[t][:, DIM + h:2 * DIM],
                                          xf[t][:, h:DIM])

            def routing(tt, ps):
                # ---- sigmoid (PSUM -> SBUF) ----
                orig = rt.tile([P, E], f32, tag="orig")
                nc.scalar.activation(orig[:], ps[:], Act.Sigmoid)

                # ---- selection scores s = orig + bias ----
                s = rt.tile([P, E], f32, tag="s")
                nc.vector.tensor_tensor(s[:], orig[:], bias_t[:], Alu.add)
                s3 = s[:].rearrange("p (g j) -> p g j", g=G)

                # ---- per-group top-2 sum ----
                m1 = rt.tile([P, G], f32, tag="m1")
                nc.vector.tensor_reduce(m1[:], s3, AxX, Alu.max)
                srep = rt.tile([P, E], f32, tag="srep")
                nc.vector.match_replace(srep[:], m1[:], s[:], NEG)
                m2 = rt.tile([P, G], f32, tag="m2")
                nc.vector.tensor_reduce(
                    m2[:], srep[:].rearrange("p (g j) -> p g j", g=G),
                    AxX, Alu.max)
                gs = rt.tile([P, G], f32, tag="gs")
                nc.vector.tensor_tensor(gs[:], m1[:], m2[:], Alu.add)

                # ---- top-4 groups: threshold = 4th largest group score ----
                gtop = rt.tile([P, 8], f32, tag="gtop")
                nc.vector.max(gtop[:], gs[:])
                km = rt.tile([P, G], f32, tag="km")  # 0 kept, NEG dropped
                nc.vector.tensor_scalar(
                    km[:], gs[:], gtop[:, TOPKG - 1:TOPKG], NEG,
                    op0=Alu.is_lt, op1=Alu.mult)

                # ---- mask dropped groups in one op (broadcast km) ----
                smask = rt.tile([P, E], f32, tag="smask")
                kmb = km[:].unsqueeze(2).broadcast_to((P, G, EPG))
                nc.vector.tensor_tensor(
                    smask[:].rearrange("p (g j) -> p g j", g=G),
                    s3, kmb, Alu.add)

                # ---- expert top-8 values; composite-key gather of the
                # original scores (critical path to w8 runs first, the
                # index extraction fills in afterwards) ----
                v8 = rt.tile([P, TOPK], f32, tag="v8")
                nc.vector.max(v8[:], smask[:])
                u = rt.tile([P, E], f32, tag="u")
                nc.vector.scalar_tensor_tensor(
                    u[:], smask[:], BIG, orig[:], op0=Alu.mult, op1=Alu.add)
                u8 = rt.tile([P, TOPK], f32, tag="u8")
                nc.vector.max(u8[:], u[:])
                o8 = rt.tile([P, TOPK], f32, tag="o8")
                nc.vector.scalar_tensor_tensor(
                    o8[:], v8[:], -BIG, u8[:], op0=Alu.mult, op1=Alu.add)

                # ---- normalize * SCALE; pack [w8|i8] into one u32 tile ----
                ssum = rt.tile([P, 1], f32, tag="ssum")
                nc.vector.tensor_reduce(ssum[:], o8[:], AxX, Alu.add)
                rec = rt.tile([P, 1], f32, tag="rec")
                nc.vector.reciprocal(rec[:], ssum[:])
                ot = rt.tile([P, 2 * TOPK], u32, tag="ot")
                nc.vector.tensor_scalar(
                    ot[:, 0:TOPK].bitcast(f32), o8[:], rec[:, 0:1], SCALE,
                    op0=Alu.mult, op1=Alu.mult)
                i8u = rt.tile([P, TOPK], u16, tag="i8u")
                nc.vector.max_index(i8u[:], v8[:], smask[:])
                nc.vector.tensor_copy(ot[:, TOPK:2 * TOPK].bitcast(i32),
                                      i8u[:])
                nc.sync.dma_start(out[tt * P:(tt + 1) * P, :], ot[:])

            wc_pair = wc_t[:].rearrange("p (two d) -> p two d", two=2)

            def corr(t):
                # fused e5m2 DoubleRow correction into the main PSUM
                lhs_all = xc[t][:].rearrange("p (two d) -> p two d", two=2)
                ps = pss[t]
                for k in range(KT):
                    nc.tensor.matmul(
                        ps[:],
                        lhs_all[:, :, k * P:(k + 1) * P],
                        wc_pair[:, :, k * E:(k + 1) * E],
                        start=False, stop=(k == KT - 1),
                        perf_mode=DRow)
                del xc[t]

            # --- software-pipelined main loop ---
            for s in range(NT + DEFER):
                if s < NT:
                    if 0 < s and s + 3 < NT:
                        tp = s + 3
                        xf[tp] = xp.tile([P, DIM], f16, tag="x", name=f"xf{tp}")
                        engs[s % 2].dma_start(xf[tp][:, 0:H], xt[tp][:, 0:H],
                                              max_dma_last_dim=1024)
                        engs[(s + 1) % 2].dma_start(xf[tp][:, H:DIM],
                                                    xt[tp][:, H:DIM],
                                                    max_dma_last_dim=1024)
                    if 0 < s + 1 < NT:
                        tq = s + 1
                        xc[tq] = xcp.tile([P, 2 * DIM], f8e5, tag="xc", name=f"xc{tq}")
                        engs[s % 2].dma_start(xc[tq][:, 0:H], xlo[tq][:, 0:H],
                                              max_dma_last_dim=2048)
                        engs[(s + 1) % 2].dma_start(xc[tq][:, H:DIM],
                                                    xlo[tq][:, H:DIM],
                                                    max_dma_last_dim=2048)
                    # cast leads the step: xf(s) is already resident
                    # (prefetched 3 tiles ahead) and corr(s) is DEFER
                    # periods away, so the DVE cast is far off the
                    # critical path
                    cast_xh5(s)
                    ps = psp.tile([P, E], f32)
                    pss[s] = ps
                    for k in range(KT):
                        nc.tensor.matmul(ps[:], xf[s][:, k * P:(k + 1) * P],
                                         wt_t[:, k * E:(k + 1) * E],
                                         start=(k == 0), stop=False)
                    del xf[s]
                if s >= DEFER:
                    t = s - DEFER
                    corr(t)
                    routing(t, pss.pop(t))

    nc.compile()
    return nc


def _shuffle_x(xc):
    """[TPC, DIM] -> [NT, P, DIM] with out[tt, p, k*128+j] = xc[tt*128+j, k*128+p]."""
    return np.ascontiguousarray(
        xc.reshape(NT, P, KT, P).transpose(0, 3, 2, 1).reshape(NT, P, DIM))


def _shuffle_w(w):
    """[E, DIM] -> [P, KT*E] with out[p, k*E+e] = w[e, k*128+p]."""
    return np.ascontiguousarray(
        w.T.reshape(KT, P, E).transpose(1, 0, 2).reshape(P, KT * E))


_nc_cache = {}


def kernel(x, weight, bias):
    import ml_dtypes

    f8 = ml_dtypes.float8_e5m2
    x = np.asarray(x, dtype=np.float32)
    weight = np.asarray(weight, dtype=np.float32)
    bias = np.asarray(bias, dtype=np.float32)

    if "nc" not in _nc_cache:
        _nc_cache["nc"] = _build()
    nc = _nc_cache["nc"]

    biasb = np.ascontiguousarray(np.broadcast_to(bias, (P, E)))
    # weight: fp16 main in the 2^6 domain; e5m2 residual in the same domain
    w6 = weight / np.float32(XS)
    wh16 = w6.astype(np.float16)
    wl5 = (w6 - wh16.astype(np.float32))
    wt_h = _shuffle_w(wh16.astype(np.float32)).astype(np.float16)
    wl_h = _shuffle_w(wl5).astype(f8)

    in_maps = []
    for c in range(NCORES):
        xcore = x[c * TPC:(c + 1) * TPC]
        xf16 = (xcore * np.float32(XS)).astype(np.float16)
        xl = xcore - xf16.astype(np.float32) / np.float32(XS)
        in_maps.append({
            "xt": _shuffle_x(xf16.astype(np.float32)).astype(np.float16),
            "xlo": _shuffle_x(xl).astype(f8),
            "wt": wt_h, "wl": wl_h, "biasb": biasb,
        })

    trace = bool(int(os.environ.get("GATE_KERNEL_TRACE", "0")))
    res = run_bass_kernel_spmd(nc, in_maps, core_ids=list(range(NCORES)),
                               trace=trace)
    last_run["exec_time_ns"] = res.exec_time_ns
    last_run["mean_exec_time_ns"] = res.mean_exec_time_ns
    last_run["trace"] = res.instructions_and_trace

    outs = [res.results[c]["out"] for c in range(NCORES)]
    buf = np.concatenate(outs, axis=0)
    w8 = buf[:, 0:TOPK].view(np.float32)
    idx = buf[:, TOPK:2 * TOPK].view(np.int32)
    return np.ascontiguousarray(w8), np.ascontiguousarray(idx)
